# revision 8
# baseline (speedup 1.0000x reference)
"""Trainium2 Bass kernel for nn_Attention (dense transformer attention block).

Reference computation (shapes fixed):
  x [2, 256, 48, 48] -> RMSNorm over channels -> 1x1 conv to qkv (8 heads, 64 dhead)
  -> prepend 4 learnable mem kv tokens -> softmax attention -> 1x1 conv out [2, 256, 48, 48]

Sharding: 8 cores = 2 batches x 4 head-pairs. Core c handles batch c//4 and
heads (2g, 2g+1), g = c%4. Each core computes its heads' attention and a
partial out-projection [256, 2304]; partials are ReduceScattered (chunked,
overlapped with compute) within each batch's 4-core group; each core returns
its 64-channel slice of the reduced output and the host reassembles.

Numerics: qkv projection in float32r (full-rate PE, ~19-bit); attention
matmuls (sim, attn@v, out-projection) in bf16 with fp32 psum accumulation.
Layout highlights:
  - x, xn in [channel, pos]; RMSNorm scale via all-ones-lhsT matmul that
    broadcasts the sum of squares to all 128 partitions.
  - q/k/v in [dhead(2 heads packed), pos]; sim matmuls row-packed (head A on
    PE rows 0-63, head B on 64-127, concurrent via row groups).
  - scores S^T [key, query] in psum; exp on ACT straight psum->sbuf; P @ v^T
    accumulated in psum with lhsT columns [ones | zeros*63 | v], giving the
    softmax denominator on partition 0 and out^T on partitions 64-127.
  - denominator: fast reciprocal (DVE) + partition broadcast (GpSimd).
"""
import numpy as np

import concourse.mybir as mybir
import concourse.tile as tile
from concourse import bacc
from concourse.bass_utils import run_bass_kernel_spmd
from concourse.masks import make_identity

F32 = mybir.dt.float32
F32R = mybir.dt.float32r
BF16 = mybir.dt.bfloat16
EXP = mybir.ActivationFunctionType.Exp
SQRT = mybir.ActivationFunctionType.Sqrt

DIM = 256
HEADS = 8
DHEAD = 64
MEM = 4
HID = 512
N = 48 * 48          # 2304 image positions
NK = N + MEM         # 2308 keys (mem tokens at the END: cols 2304:2308)
NJT = N // 128       # 18 image j-tiles
GROUPS = [[0, 1, 2, 3], [4, 5, 6, 7]]

# i-chunks of the query axis
CHUNKS = [(0, 512), (512, 512), (1024, 512), (1536, 512), (2048, 256)]


def build():
    nc = bacc.Bacc("TRN2", target_bir_lowering=False, debug=False,
                   enable_asserts=True, num_devices=8)
    x_d = nc.dram_tensor("x", [DIM, N], F32, kind="ExternalInput").ap()
    wqkv_d = nc.dram_tensor("wqkv", [DIM, 384], F32, kind="ExternalInput").ap()
    memk_d = nc.dram_tensor("memk", [128, MEM], F32, kind="ExternalInput").ap()
    memv_d = nc.dram_tensor("memv", [MEM, 2, DHEAD], F32, kind="ExternalInput").ap()
    woutT_d = nc.dram_tensor("woutT", [2, DHEAD, DIM], F32, kind="ExternalInput").ap()
    out_d = nc.dram_tensor("out", [DHEAD, N], F32, kind="ExternalOutput").ap()

    NCH = len(CHUNKS)
    with tile.TileContext(nc) as tc:
        with (
            tc.tile_pool(name="consts", bufs=1) as consts,
            tc.tile_pool(name="big", bufs=1) as big,
            tc.tile_pool(name="io", bufs=2) as io,
            tc.tile_pool(name="pP", bufs=3) as pP,
            tc.tile_pool(name="ps_s", bufs=2, space="PSUM") as ps_s,
            tc.tile_pool(name="ps_a", bufs=2, space="PSUM") as ps_a,
            tc.tile_pool(name="dram", bufs=1, space="DRAM") as dram,
        ):
            # ---------------- constants ----------------
            ident = consts.tile([128, 128], F32)
            make_identity(nc, ident)
            ones_f = consts.tile([128, 1], F32)
            nc.vector.memset(ones_f[:, :], 1.0)
            zeros_f = consts.tile([128, 1], F32)
            nc.vector.memset(zeros_f[:, :], 0.0)
            ones_r = consts.tile([128, 128], F32R)
            nc.vector.tensor_copy(ones_r[:, :], ones_f[:, :].to_broadcast((128, 128)))

            # ---------------- collective warmup ----------------
            # the first collective on a NEFF pays ~60us of firmware cold
            # start; absorb it behind the compute phase with a tiny dummy.
            warm_sb = consts.tile([1, 32], F32)
            nc.vector.memset(warm_sb[:, :], 0.0)
            wi = dram.tile([1, 32], F32, tag="wi")
            wo = dram.tile([1, 32], F32, tag="wo")
            nc.sync.dma_start(out=wi[:, :], in_=warm_sb[:, :])
            nc.gpsimd.collective_compute(
                "AllReduce", mybir.AluOpType.add,
                replica_groups=GROUPS,
                ins=[wi[:, :].opt()],
                outs=[wo[:, :].opt()],
            )

            # ---------------- vT tile skeletons (no data deps) ----------------
            # per (head, jt): [key(128 part), 128]: col 0 ones, 1:64 zeros,
            # 64:128 v^T.  jt == NJT holds the 4 mem tokens on rows 0:4.
            vT = [[None, None] for _ in range(NJT + 1)]
            for jt in range(NJT + 1):
                for h in range(2):
                    t = big.tile([128, 128], BF16, tag=f"vT{h}_{jt}")
                    vT[jt][h] = t
                    nc.vector.tensor_copy(
                        t[:, 0:1], ones_f[:, :].to_broadcast((128, 1)))
                    nc.vector.tensor_copy(
                        t[:, 1:64], zeros_f[:, :].to_broadcast((128, 63)))

            # ---------------- load inputs ----------------
            xs = [[None] * NCH, [None] * NCH]
            for ci, (c0, cw) in enumerate(CHUNKS):
                for kt in range(2):
                    t = big.tile([128, cw], F32, tag=f"x{kt}_{ci}")
                    xs[kt][ci] = t
                    nc.sync.dma_start(
                        out=t[:, :], in_=x_d[128 * kt:128 * kt + 128, c0:c0 + cw])

            wq_f = io.tile([128, 2, 384], F32)
            nc.sync.dma_start(out=wq_f[:, 0, :], in_=wqkv_d[0:128, :])
            nc.sync.dma_start(out=wq_f[:, 1, :], in_=wqkv_d[128:256, :])
            wq = consts.tile([128, 2, 384], F32R)
            nc.vector.tensor_copy(wq[:, :, :], wq_f[:, :, :])

            memk_f = io.tile([128, MEM], F32)
            nc.sync.dma_start(out=memk_f[:, :], in_=memk_d)
            kmem = consts.tile([128, MEM], BF16)
            nc.vector.tensor_copy(kmem[:, :], memk_f[:, :])
            memv_f = io.tile([MEM, 2, DHEAD], F32)
            nc.sync.dma_start(out=memv_f[:, :, :], in_=memv_d)
            for h in range(2):
                nc.vector.tensor_copy(vT[NJT][h][0:MEM, 64:128], memv_f[:, h, :])

            # wout lhsT tiles, one per head, data on partitions 64..127
            woutA_f = io.tile([128, DIM], F32, tag="woutA_f")
            woutB_f = io.tile([128, DIM], F32, tag="woutB_f")
            nc.sync.dma_start(out=woutA_f[64:128, :], in_=woutT_d[0, :, :])
            nc.sync.dma_start(out=woutB_f[64:128, :], in_=woutT_d[1, :, :])
            woutA = consts.tile([128, DIM], BF16, tag="woutA")
            woutB = consts.tile([128, DIM], BF16, tag="woutB")
            nc.vector.tensor_copy(woutA[64:128, :], woutA_f[64:128, :])
            nc.vector.tensor_copy(woutB[64:128, :], woutB_f[64:128, :])
            wouts = [woutA, woutB]

            # ------------- per-chunk: RMSNorm + qkv + v^T -------------
            SQUARE = mybir.ActivationFunctionType.Square
            qs, ks, vs = [], [], []
            for ci, (c0, cw) in enumerate(CHUNKS):
                xsq0 = pP.tile([128, 512], F32R, tag="xsq0")
                xsq1 = pP.tile([128, 512], F32R, tag="xsq1")
                nc.scalar.activation(xsq0[:, 0:cw], xs[0][ci][:, :], SQUARE)
                nc.scalar.activation(xsq1[:, 0:cw], xs[1][ci][:, :], SQUARE)
                sb_ps = ps_s.tile([128, 2, 512], F32, tag="s")
                nc.tensor.matmul(sb_ps[:, 0, 0:cw], ones_r[:, :],
                                 xsq0[:, 0:cw], start=True, stop=False)
                nc.tensor.matmul(sb_ps[:, 0, 0:cw], ones_r[:, :],
                                 xsq1[:, 0:cw], start=False, stop=True)
                sinv = pP.tile([128, 512], F32, tag="sinv")
                nc.scalar.activation(sinv[:, 0:cw], sb_ps[:, 0, 0:cw], SQRT,
                                     scale=1.0 / 256.0)
                nc.vector.reciprocal_approx_fast(sinv[:, 0:cw], sinv[:, 0:cw])
                xn0 = pP.tile([128, 512], F32R, tag="xn0")
                xn1 = pP.tile([128, 512], F32R, tag="xn1")
                nc.vector.tensor_mul(xn0[:, 0:cw], xs[0][ci][:, :], sinv[:, 0:cw])
                nc.vector.tensor_mul(xn1[:, 0:cw], xs[1][ci][:, :], sinv[:, 0:cw])
                xns = [xn0, xn1]

                qc = big.tile([128, cw], BF16, tag=f"q{ci}")
                kc = big.tile([128, cw], BF16, tag=f"k{ci}")
                vc = big.tile([128, cw], F32, tag=f"v{ci}")
                qs.append(qc)
                ks.append(kc)
                vs.append(vc)
                for m, dst in ((0, qc), (1, kc), (2, vc)):
                    qp = ps_a.tile([128, 512], F32, tag="a0")
                    for kt in range(2):
                        nc.tensor.matmul(
                            qp[:, 0:cw],
                            wq[:, kt, m * 128:(m + 1) * 128],
                            xns[kt][:, 0:cw],
                            start=(kt == 0), stop=(kt == 1),
                        )
                    if m == 1:
                        nc.scalar.copy(dst[:, :], qp[:, 0:cw])
                    else:
                        nc.vector.tensor_copy(dst[:, :], qp[:, 0:cw])
                # transposes for this chunk's j-tiles
                for jl in range(cw // 128):
                    jt = c0 // 128 + jl
                    for h in range(2):
                        tp = ps_a.tile([128, 64], F32, tag="a1")
                        nc.tensor.transpose(
                            tp[:, :],
                            vc[64 * h:64 * h + 64, jl * 128:(jl + 1) * 128],
                            ident[64 * h:64 * h + 64, 64 * h:64 * h + 64],
                        )
                        nc.vector.tensor_copy(vT[jt][h][:, 64:128], tp[:, :])

            # ---------------- attention + out projection ----------------
            rec = io.tile([1, 2, 512], F32, tag="rec")
            for ci, (c0, cw) in enumerate(CHUNKS):
                acc0 = ps_a.tile([128, 512], F32, tag="a0")
                acc1 = ps_a.tile([128, 512], F32, tag="a1")
                accs = [acc0, acc1]
                for jt in range(NJT + 1):
                    s_ps = ps_s.tile([128, 2, 512], F32, tag="s")
                    if jt < NJT:
                        km = 128
                        klhs = [ks[jt // 4][64 * h:64 * h + 64,
                                            (jt % 4) * 128:(jt % 4) * 128 + 128]
                                for h in range(2)]
                    else:
                        km = MEM
                        klhs = [kmem[64 * h:64 * h + 64, :] for h in range(2)]
                    for h in range(2):
                        nc.tensor.matmul(
                            s_ps[0:km, h, 0:cw],
                            klhs[h],
                            qs[ci][64 * h:64 * h + 64, :],
                            start=True, stop=True,
                        )
                    P = pP.tile([128, 2, 512], BF16, tag="P")
                    nc.scalar.activation(P[0:km, :, 0:cw], s_ps[0:km, :, 0:cw], EXP)
                    for h in range(2):
                        nc.tensor.matmul(
                            accs[h][:, 0:cw],
                            vT[jt][h][0:km, :],
                            P[0:km, h, 0:cw],
                            start=(jt == 0), stop=(jt == NJT),
                            skip_group_check=True,
                        )
                # normalize: out^T_h = acc[64:128] * (1/acc[0])
                rb = pP.tile([128, 2, 512], F32, tag="rb")
                for h in range(2):
                    nc.vector.reciprocal_approx_fast(
                        rec[0:1, h, 0:cw], accs[h][0:1, 0:cw])
                    nc.gpsimd.partition_broadcast(rb[:, h, 0:cw], rec[0:1, h, 0:cw])
                oT0 = pP.tile([128, 512], BF16, tag="oT0")
                oT1 = pP.tile([128, 512], BF16, tag="oT1")
                oTs = [oT0, oT1]
                for h in range(2):
                    nc.vector.tensor_mul(
                        oTs[h][64:128, 0:cw], accs[h][64:128, 0:cw],
                        rb[64:128, h, 0:cw])
                # out projection: [256, cw] partial = sum_h woutT_h.T @ oT_h
                osb = pP.tile([128, 2, 512], F32, tag="osb")
                for mt in range(2):
                    op = ps_a.tile([128, 512], F32, tag=f"a{mt}")
                    for h in range(2):
                        nc.tensor.matmul(
                            op[:, 0:cw],
                            wouts[h][64:128, mt * 128:(mt + 1) * 128],
                            oTs[h][64:128, 0:cw],
                            start=(h == 0), stop=(h == 1),
                        )
                    nc.vector.tensor_copy(osb[:, mt, 0:cw], op[:, 0:cw])
                # chunked reduce-scatter of the [256, cw] partial
                bi = dram.tile([2, 128, cw], F32, tag=f"bi{ci}")
                bo = dram.tile([DHEAD, cw], F32, tag=f"bo{ci}")
                nc.sync.dma_start(out=bi[0, :, :], in_=osb[:, 0, 0:cw])
                nc.sync.dma_start(out=bi[1, :, :], in_=osb[:, 1, 0:cw])
                nc.gpsimd.collective_compute(
                    "ReduceScatter", mybir.AluOpType.add,
                    replica_groups=GROUPS,
                    ins=[bi[:, :, :].opt()],
                    outs=[bo[:, :].opt()],
                )
                nc.sync.dma_start(out=out_d[:, c0:c0 + cw], in_=bo[:, :])
    nc.compile()
    return nc


_NC = None
_last_in_maps = None


def _get_nc():
    global _NC
    if _NC is None:
        _NC = build()
    return _NC


def make_in_maps(x, gamma, mem_kv, w_qkv, w_out):
    x = np.asarray(x, np.float32)
    gamma = np.asarray(gamma, np.float32).reshape(DIM)
    mem_kv = np.asarray(mem_kv, np.float32)
    w_qkv = np.asarray(w_qkv, np.float32)
    w_out = np.asarray(w_out, np.float32)

    g1 = 1.0 + gamma  # [256]
    scale = DHEAD ** -0.5
    in_maps = []
    for core in range(8):
        b, g = core // 4, core % 4
        hA, hB = 2 * g, 2 * g + 1
        blocks = []
        for t in range(3):  # q, k, v
            for h in (hA, hB):
                wblk = w_qkv[t * HID + h * DHEAD: t * HID + (h + 1) * DHEAD, :]
                if t == 0:
                    wblk = wblk * scale
                blocks.append(wblk.T)  # [256, 64]
        wqkvT = np.concatenate(blocks, axis=1) * g1[:, None]  # [256, 384]
        memk = np.concatenate(
            [mem_kv[0, hA].T, mem_kv[0, hB].T], axis=0)  # [128, 4]
        memv = np.stack([mem_kv[1, hA], mem_kv[1, hB]], axis=1)  # [4, 2, 64]
        woutT = np.stack(
            [w_out[:, hA * DHEAD:(hA + 1) * DHEAD].T,
             w_out[:, hB * DHEAD:(hB + 1) * DHEAD].T], axis=0)  # [2, 64, 256]
        in_maps.append({
            "x": np.ascontiguousarray(x[b].reshape(DIM, N)),
            "wqkv": np.ascontiguousarray(wqkvT),
            "memk": np.ascontiguousarray(memk),
            "memv": np.ascontiguousarray(memv),
            "woutT": np.ascontiguousarray(woutT),
        })
    return in_maps


def kernel(x, gamma, mem_kv, w_qkv, w_out):
    global _last_in_maps
    in_maps = make_in_maps(x, gamma, mem_kv, w_qkv, w_out)
    _last_in_maps = in_maps
    nc = _get_nc()
    res = run_bass_kernel_spmd(nc, in_maps, core_ids=list(range(8)))
    out = np.empty((2, DIM, N), np.float32)
    for core in range(8):
        b, g = core // 4, core % 4
        out[b, 64 * g:64 * g + 64, :] = res.results[core]["out"]
    return out.reshape(2, DIM, 48, 48)


# revision 9
# speedup vs baseline: 1.0411x; 1.0411x over previous
"""Trainium2 Bass kernel for nn_Attention (dense transformer attention block).

Reference computation (shapes fixed):
  x [2, 256, 48, 48] -> RMSNorm over channels -> 1x1 conv to qkv (8 heads, 64 dhead)
  -> prepend 4 learnable mem kv tokens -> softmax attention -> 1x1 conv out [2, 256, 48, 48]

Sharding: 8 cores = 2 batches x 4 head-pairs. Core c handles batch c//4 and
heads (2g, 2g+1), g = c%4. Each core computes its heads' attention and a
partial out-projection [256, 2304]; partials are ReduceScattered (chunked,
overlapped with compute) within each batch's 4-core group; each core returns
its 64-channel slice of the reduced output and the host reassembles.

Numerics: qkv projection in float32r (full-rate PE, ~19-bit); attention
matmuls (sim, attn@v, out-projection) in bf16 with fp32 psum accumulation.
Layout highlights:
  - x, xn in [channel, pos]; RMSNorm scale via all-ones-lhsT matmul that
    broadcasts the sum of squares to all 128 partitions.
  - q/k/v in [dhead(2 heads packed), pos]; sim matmuls row-packed (head A on
    PE rows 0-63, head B on 64-127, concurrent via row groups).
  - scores S^T [key, query] in psum; exp on ACT straight psum->sbuf; P @ v^T
    accumulated in psum with lhsT columns [ones | zeros*63 | v], giving the
    softmax denominator on partition 0 and out^T on partitions 64-127.
  - denominator: fast reciprocal (DVE) + partition broadcast (GpSimd).
"""
import numpy as np

import concourse.mybir as mybir
import concourse.tile as tile
from concourse import bacc
from concourse.bass_utils import run_bass_kernel_spmd
from concourse.masks import make_identity

F32 = mybir.dt.float32
F32R = mybir.dt.float32r
BF16 = mybir.dt.bfloat16
EXP = mybir.ActivationFunctionType.Exp
SQRT = mybir.ActivationFunctionType.Sqrt

DIM = 256
HEADS = 8
DHEAD = 64
MEM = 4
HID = 512
N = 48 * 48          # 2304 image positions
NK = N + MEM         # 2308 keys (mem tokens at the END: cols 2304:2308)
NJT = N // 128       # 18 image j-tiles
GROUPS = [[0, 1, 2, 3], [4, 5, 6, 7]]

# i-chunks of the query axis
CHUNKS = [(0, 512), (512, 512), (1024, 512), (1536, 512), (2048, 256)]


def build():
    nc = bacc.Bacc("TRN2", target_bir_lowering=False, debug=False,
                   enable_asserts=True, num_devices=8)
    x_d = nc.dram_tensor("x", [DIM, N], F32, kind="ExternalInput").ap()
    wqkv_d = nc.dram_tensor("wqkv", [DIM, 384], F32, kind="ExternalInput").ap()
    memk_d = nc.dram_tensor("memk", [128, MEM], F32, kind="ExternalInput").ap()
    memv_d = nc.dram_tensor("memv", [MEM, 2, DHEAD], F32, kind="ExternalInput").ap()
    woutT_d = nc.dram_tensor("woutT", [2, DHEAD, DIM], F32, kind="ExternalInput").ap()
    out_d = nc.dram_tensor("out", [DHEAD, N], F32, kind="ExternalOutput").ap()

    NCH = len(CHUNKS)
    with tile.TileContext(nc) as tc:
        with (
            tc.tile_pool(name="consts", bufs=1) as consts,
            tc.tile_pool(name="big", bufs=1) as big,
            tc.tile_pool(name="io", bufs=2) as io,
            tc.tile_pool(name="pP", bufs=3) as pP,
            tc.tile_pool(name="ps_s", bufs=2, space="PSUM") as ps_s,
            tc.tile_pool(name="ps_a", bufs=2, space="PSUM") as ps_a,
            tc.tile_pool(name="dram", bufs=1, space="DRAM") as dram,
        ):
            # ---------------- constants ----------------
            ident = consts.tile([128, 128], F32)
            make_identity(nc, ident)
            ones_f = consts.tile([128, 1], F32)
            nc.vector.memset(ones_f[:, :], 1.0)
            zeros_f = consts.tile([128, 1], F32)
            nc.vector.memset(zeros_f[:, :], 0.0)
            ones_r = consts.tile([128, 128], F32R)
            nc.vector.tensor_copy(ones_r[:, :], ones_f[:, :].to_broadcast((128, 128)))

            # ---------------- collective warmup ----------------
            # the first collective on a NEFF pays ~60us of firmware cold
            # start; absorb it behind the compute phase with a tiny dummy.
            warm_sb = consts.tile([1, 32], F32)
            nc.vector.memset(warm_sb[:, :], 0.0)
            wi = dram.tile([1, 32], F32, tag="wi")
            wo = dram.tile([1, 32], F32, tag="wo")
            nc.sync.dma_start(out=wi[:, :], in_=warm_sb[:, :])
            nc.gpsimd.collective_compute(
                "AllReduce", mybir.AluOpType.add,
                replica_groups=GROUPS,
                ins=[wi[:, :].opt()],
                outs=[wo[:, :].opt()],
            )

            # ---------------- vT tile skeletons (no data deps) ----------------
            # per (head, jt): [key(128 part), 128]: col 0 ones, 1:64 zeros,
            # 64:128 v^T.  jt == NJT holds the 4 mem tokens on rows 0:4.
            vT = [[None, None] for _ in range(NJT + 1)]
            for jt in range(NJT + 1):
                for h in range(2):
                    t = big.tile([128, 128], BF16, tag=f"vT{h}_{jt}")
                    vT[jt][h] = t
                    nc.vector.tensor_copy(
                        t[:, 0:1], ones_f[:, :].to_broadcast((128, 1)))
                    nc.vector.tensor_copy(
                        t[:, 1:64], zeros_f[:, :].to_broadcast((128, 63)))

            # ---------------- load inputs ----------------
            xs = [[None] * NCH, [None] * NCH]
            for ci, (c0, cw) in enumerate(CHUNKS):
                for kt in range(2):
                    t = big.tile([128, cw], F32, tag=f"x{kt}_{ci}")
                    xs[kt][ci] = t
                    nc.sync.dma_start(
                        out=t[:, :], in_=x_d[128 * kt:128 * kt + 128, c0:c0 + cw])

            wq_f = io.tile([128, 2, 384], F32)
            nc.sync.dma_start(out=wq_f[:, 0, :], in_=wqkv_d[0:128, :])
            nc.sync.dma_start(out=wq_f[:, 1, :], in_=wqkv_d[128:256, :])
            wq = consts.tile([128, 2, 384], F32R)
            nc.vector.tensor_copy(wq[:, :, :], wq_f[:, :, :])

            memk_f = io.tile([128, MEM], F32)
            nc.sync.dma_start(out=memk_f[:, :], in_=memk_d)
            kmem = consts.tile([128, MEM], BF16)
            nc.vector.tensor_copy(kmem[:, :], memk_f[:, :])
            memv_f = io.tile([MEM, 2, DHEAD], F32)
            nc.sync.dma_start(out=memv_f[:, :, :], in_=memv_d)
            for h in range(2):
                nc.vector.tensor_copy(vT[NJT][h][0:MEM, 64:128], memv_f[:, h, :])

            # wout lhsT tiles, one per head, data on partitions 64..127
            woutA_f = io.tile([128, DIM], F32, tag="woutA_f")
            woutB_f = io.tile([128, DIM], F32, tag="woutB_f")
            nc.sync.dma_start(out=woutA_f[64:128, :], in_=woutT_d[0, :, :])
            nc.sync.dma_start(out=woutB_f[64:128, :], in_=woutT_d[1, :, :])
            woutA = consts.tile([128, DIM], BF16, tag="woutA")
            woutB = consts.tile([128, DIM], BF16, tag="woutB")
            nc.vector.tensor_copy(woutA[64:128, :], woutA_f[64:128, :])
            nc.vector.tensor_copy(woutB[64:128, :], woutB_f[64:128, :])
            wouts = [woutA, woutB]

            # ------------- per-chunk: RMSNorm + qkv + v^T -------------
            SQUARE = mybir.ActivationFunctionType.Square
            qs, ks, vs = [], [], []
            for ci, (c0, cw) in enumerate(CHUNKS):
                xsq0 = pP.tile([128, 512], F32R, tag="xsq0")
                xsq1 = pP.tile([128, 512], F32R, tag="xsq1")
                nc.scalar.activation(xsq0[:, 0:cw], xs[0][ci][:, :], SQUARE)
                nc.scalar.activation(xsq1[:, 0:cw], xs[1][ci][:, :], SQUARE)
                sb_ps = ps_s.tile([128, 2, 512], F32, tag="s")
                nc.tensor.matmul(sb_ps[:, 0, 0:cw], ones_r[:, :],
                                 xsq0[:, 0:cw], start=True, stop=False)
                nc.tensor.matmul(sb_ps[:, 0, 0:cw], ones_r[:, :],
                                 xsq1[:, 0:cw], start=False, stop=True)
                sinv = pP.tile([128, 512], F32, tag="sinv")
                nc.scalar.activation(sinv[:, 0:cw], sb_ps[:, 0, 0:cw], SQRT,
                                     scale=1.0 / 256.0)
                nc.vector.reciprocal_approx_fast(sinv[:, 0:cw], sinv[:, 0:cw])
                xn0 = pP.tile([128, 512], F32R, tag="xn0")
                xn1 = pP.tile([128, 512], F32R, tag="xn1")
                nc.vector.tensor_mul(xn0[:, 0:cw], xs[0][ci][:, :], sinv[:, 0:cw])
                nc.vector.tensor_mul(xn1[:, 0:cw], xs[1][ci][:, :], sinv[:, 0:cw])
                xns = [xn0, xn1]

                qc = big.tile([128, cw], BF16, tag=f"q{ci}")
                kc = big.tile([128, cw], BF16, tag=f"k{ci}")
                vc = big.tile([128, cw], F32, tag=f"v{ci}")
                qs.append(qc)
                ks.append(kc)
                vs.append(vc)
                for m, dst in ((0, qc), (1, kc), (2, vc)):
                    qp = ps_a.tile([128, 512], F32, tag="a0")
                    for kt in range(2):
                        nc.tensor.matmul(
                            qp[:, 0:cw],
                            wq[:, kt, m * 128:(m + 1) * 128],
                            xns[kt][:, 0:cw],
                            start=(kt == 0), stop=(kt == 1),
                        )
                    if m == 1:
                        nc.scalar.copy(dst[:, :], qp[:, 0:cw])
                    else:
                        nc.vector.tensor_copy(dst[:, :], qp[:, 0:cw])
                # transposes for this chunk's j-tiles
                for jl in range(cw // 128):
                    jt = c0 // 128 + jl
                    for h in range(2):
                        tp = ps_a.tile([128, 64], F32, tag="a1")
                        nc.tensor.transpose(
                            tp[:, :],
                            vc[64 * h:64 * h + 64, jl * 128:(jl + 1) * 128],
                            ident[64 * h:64 * h + 64, 64 * h:64 * h + 64],
                        )
                        nc.vector.tensor_copy(vT[jt][h][:, 64:128], tp[:, :])

            # ---------------- attention + out projection ----------------
            bi0 = dram.tile([2, 128, 1024], F32, tag="bi0")
            bi1 = dram.tile([2, 128, 1280], F32, tag="bi1")
            bo0 = dram.tile([DHEAD, 1024], F32, tag="bo0")
            bo1 = dram.tile([DHEAD, 1280], F32, tag="bo1")
            bis = [bi0, bi1]
            bos = [bo0, bo1]
            rec = io.tile([1, 2, 512], F32, tag="rec")
            for ci, (c0, cw) in enumerate(CHUNKS):
                acc0 = ps_a.tile([128, 512], F32, tag="a0")
                acc1 = ps_a.tile([128, 512], F32, tag="a1")
                accs = [acc0, acc1]
                for jt in range(NJT + 1):
                    s_ps = ps_s.tile([128, 2, 512], F32, tag="s")
                    if jt < NJT:
                        km = 128
                        klhs = [ks[jt // 4][64 * h:64 * h + 64,
                                            (jt % 4) * 128:(jt % 4) * 128 + 128]
                                for h in range(2)]
                    else:
                        km = MEM
                        klhs = [kmem[64 * h:64 * h + 64, :] for h in range(2)]
                    for h in range(2):
                        nc.tensor.matmul(
                            s_ps[0:km, h, 0:cw],
                            klhs[h],
                            qs[ci][64 * h:64 * h + 64, :],
                            start=True, stop=True,
                        )
                    P = pP.tile([128, 2, 512], BF16, tag="P")
                    nc.scalar.activation(P[0:km, :, 0:cw], s_ps[0:km, :, 0:cw], EXP)
                    for h in range(2):
                        nc.tensor.matmul(
                            accs[h][:, 0:cw],
                            vT[jt][h][0:km, :],
                            P[0:km, h, 0:cw],
                            start=(jt == 0), stop=(jt == NJT),
                            skip_group_check=True,
                        )
                # normalize: out^T_h = acc[64:128] * (1/acc[0])
                rb = pP.tile([128, 2, 512], F32, tag="rb")
                for h in range(2):
                    nc.vector.reciprocal_approx_fast(
                        rec[0:1, h, 0:cw], accs[h][0:1, 0:cw])
                    nc.gpsimd.partition_broadcast(rb[:, h, 0:cw], rec[0:1, h, 0:cw])
                oT0 = pP.tile([128, 512], BF16, tag="oT0")
                oT1 = pP.tile([128, 512], BF16, tag="oT1")
                oTs = [oT0, oT1]
                for h in range(2):
                    nc.vector.tensor_mul(
                        oTs[h][64:128, 0:cw], accs[h][64:128, 0:cw],
                        rb[64:128, h, 0:cw])
                # out projection: [256, cw] partial = sum_h woutT_h.T @ oT_h
                osb = pP.tile([128, 2, 512], F32, tag="osb")
                for mt in range(2):
                    op = ps_a.tile([128, 512], F32, tag=f"a{mt}")
                    for h in range(2):
                        nc.tensor.matmul(
                            op[:, 0:cw],
                            wouts[h][64:128, mt * 128:(mt + 1) * 128],
                            oTs[h][64:128, 0:cw],
                            start=(h == 0), stop=(h == 1),
                        )
                    nc.vector.tensor_copy(osb[:, mt, 0:cw], op[:, 0:cw])
                # stage the [256, cw] partial into the half's bounce buffer
                half = 0 if ci < 2 else 1
                h0 = 0 if half == 0 else 1024
                bi = bis[half]
                nc.sync.dma_start(out=bi[0, :, c0 - h0:c0 - h0 + cw],
                                  in_=osb[:, 0, 0:cw])
                nc.sync.dma_start(out=bi[1, :, c0 - h0:c0 - h0 + cw],
                                  in_=osb[:, 1, 0:cw])
                if ci in (1, 4):
                    nc.gpsimd.collective_compute(
                        "ReduceScatter", mybir.AluOpType.add,
                        replica_groups=GROUPS,
                        ins=[bi[:, :, :].opt()],
                        outs=[bos[half][:, :].opt()],
                    )
            nc.sync.dma_start(out=out_d[:, 0:1024], in_=bos[0][:, :])
            nc.sync.dma_start(out=out_d[:, 1024:N], in_=bos[1][:, :])
    nc.compile()
    return nc


_NC = None
_last_in_maps = None


def _get_nc():
    global _NC
    if _NC is None:
        _NC = build()
    return _NC


def make_in_maps(x, gamma, mem_kv, w_qkv, w_out):
    x = np.asarray(x, np.float32)
    gamma = np.asarray(gamma, np.float32).reshape(DIM)
    mem_kv = np.asarray(mem_kv, np.float32)
    w_qkv = np.asarray(w_qkv, np.float32)
    w_out = np.asarray(w_out, np.float32)

    g1 = 1.0 + gamma  # [256]
    scale = DHEAD ** -0.5
    in_maps = []
    for core in range(8):
        b, g = core // 4, core % 4
        hA, hB = 2 * g, 2 * g + 1
        blocks = []
        for t in range(3):  # q, k, v
            for h in (hA, hB):
                wblk = w_qkv[t * HID + h * DHEAD: t * HID + (h + 1) * DHEAD, :]
                if t == 0:
                    wblk = wblk * scale
                blocks.append(wblk.T)  # [256, 64]
        wqkvT = np.concatenate(blocks, axis=1) * g1[:, None]  # [256, 384]
        memk = np.concatenate(
            [mem_kv[0, hA].T, mem_kv[0, hB].T], axis=0)  # [128, 4]
        memv = np.stack([mem_kv[1, hA], mem_kv[1, hB]], axis=1)  # [4, 2, 64]
        woutT = np.stack(
            [w_out[:, hA * DHEAD:(hA + 1) * DHEAD].T,
             w_out[:, hB * DHEAD:(hB + 1) * DHEAD].T], axis=0)  # [2, 64, 256]
        in_maps.append({
            "x": np.ascontiguousarray(x[b].reshape(DIM, N)),
            "wqkv": np.ascontiguousarray(wqkvT),
            "memk": np.ascontiguousarray(memk),
            "memv": np.ascontiguousarray(memv),
            "woutT": np.ascontiguousarray(woutT),
        })
    return in_maps


def kernel(x, gamma, mem_kv, w_qkv, w_out):
    global _last_in_maps
    in_maps = make_in_maps(x, gamma, mem_kv, w_qkv, w_out)
    _last_in_maps = in_maps
    nc = _get_nc()
    res = run_bass_kernel_spmd(nc, in_maps, core_ids=list(range(8)))
    out = np.empty((2, DIM, N), np.float32)
    for core in range(8):
        b, g = core // 4, core % 4
        out[b, 64 * g:64 * g + 64, :] = res.results[core]["out"]
    return out.reshape(2, DIM, 48, 48)
